# revision 34
# baseline (speedup 1.0000x reference)
"""MaxPoolAggregator GNN kernel for 8 Trainium2 NeuronCores.

Reference computation:
    H = relu(X[trg] @ fc_w + fc_b)  per edge           [E, 512]
    agg = clamp0(segment_max(H, src))                  [N, 512]
    out = concat([X, agg], 1) @ weights_matrix         [N, 128]

Strategy (src-partitioned, no cross-core reduction):
  - Each core owns a contiguous range of 6272 src nodes and all their edges.
  - Host sorts edges by src, buckets each node by next_pow2(max(lo_deg, hi_deg))
    where lo/hi split the target-id space at 25088 (dma_gather int16 limit).
  - Per 512-slot subtile: dma_gather(transpose) pulls bf16 X rows of the edge
    targets directly in feature-major layout; PE matmuls against fc_w chunks
    (bf16, f32 PSUM); the grouped segment max runs on DVE (tensor_reduce from
    PSUM) or ACT-copy + DVE bf16 max-tree (drain bandwidth split).
  - Pads duplicate a real neighbor (max-idempotent); phase-empty groups are
    knocked out with a -1e30 column offset before the cross-phase max merge.
  - max commutes with relu and the +bias is per-feature, so bias+relu runs
    once per node on ACT after the reduce; empty nodes relu(-1e30+b) = 0,
    matching the reference's zero-fill clamp.
  - Final: out = [X^T; agg^T]^T @ wout as 5 accumulated K=128 matmuls per
    128-node chunk, written node-major straight to DRAM.
"""
import sys
import os

sys.path.insert(0, "/opt/trn_rl_repo")

import numpy as np
import ml_dtypes

N_NODES = 50000
N_EDGES = 800000
D_IN = 128
D_HID = 512
D_OUT = 128
NCORES = 8
NPC = 6272            # nodes per core (50176 padded / 8)
SPLIT = 25088         # target-range split for int16 gather indices
SUB = 512             # slots per gather/reduce subtile
BUCKETS = [1, 2, 4, 8, 16, 32, 64, 128, 256, 512]
NEG = -1.0e30

_compiled = None
LAST_RES = None


def _build_host_structures(adjacency):
    """Sort edges by src, bucket nodes, build per-core slot/index streams."""
    src = np.asarray(adjacency[0], dtype=np.int64)
    trg = np.asarray(adjacency[1], dtype=np.int64)
    order = np.argsort(src, kind="stable")
    src_s = src[order]
    trg_s = trg[order]
    deg = np.bincount(src, minlength=N_NODES).astype(np.int64)
    rowptr = np.zeros(N_NODES + 1, np.int64)
    np.cumsum(deg, out=rowptr[1:])
    # per-node lo/hi degree (trg < SPLIT vs >=)
    is_lo = (trg_s < SPLIT).astype(np.int64)
    lo_cum = np.zeros(N_EDGES + 1, np.int64)
    np.cumsum(is_lo, out=lo_cum[1:])
    d_lo = lo_cum[rowptr[1:]] - lo_cum[rowptr[:-1]]
    d_hi = deg - d_lo
    assert deg.max() <= 512, f"degree {deg.max()} exceeds supported 512"

    dmax = np.maximum(np.maximum(d_lo, d_hi), 1)
    bidx = np.searchsorted(BUCKETS, dmax, side="left")  # BUCKETS[bidx] >= dmax

    cores = []
    # per-bucket subtile counts must be equal across cores (SPMD): take max
    per_core_nodes = [np.arange(k * NPC, min((k + 1) * NPC, N_NODES)) for k in range(NCORES)]
    n_sub_per_bucket = []
    for gi, g in enumerate(BUCKETS):
        counts = [(bidx[nodes] == gi).sum() for nodes in per_core_nodes]
        n_sub_per_bucket.append(
            max((int(c) * g + SUB - 1) // SUB for c in counts) if max(counts) else 0
        )
    n_sub = sum(n_sub_per_bucket)
    # subtile -> bucket size table (same for all cores)
    sub_g = []
    for gi, g in enumerate(BUCKETS):
        sub_g += [g] * n_sub_per_bucket[gi]
    cap = sum(SUB // g for g in sub_g)          # total output columns
    capP = ((cap + 127) // 128) * 128           # padded for final matmul

    for k in range(NCORES):
        nodes = per_core_nodes[k]
        col_node = np.full(capP, -1, np.int64)  # column -> global node id
        # slot target streams per phase
        slots = {0: np.zeros(n_sub * SUB, np.int64), 1: np.zeros(n_sub * SUB, np.int64)}
        off = np.zeros((2, capP), np.float32)   # -1e30 for empty/trash columns
        off[:, :] = NEG                          # default: trash
        col = 0
        slot = 0
        for gi, g in enumerate(BUCKETS):
            if n_sub_per_bucket[gi] == 0:
                continue
            sel = nodes[bidx[nodes] == gi]
            cap_groups = n_sub_per_bucket[gi] * (SUB // g)
            for i in range(cap_groups):
                if i < len(sel):
                    n = int(sel[i])
                    col_node[col] = n
                    lo0, hi0 = rowptr[n], rowptr[n + 1]
                    tt = trg_s[lo0:hi0]
                    tlo = tt[tt < SPLIT]
                    thi = tt[tt >= SPLIT] - SPLIT
                    for ph, tp in ((0, tlo), (1, thi)):
                        if len(tp):
                            arr = np.empty(g, np.int64)
                            arr[: len(tp)] = tp
                            arr[len(tp):] = tp[0]
                            off[ph, col] = 0.0
                        else:
                            arr = np.zeros(g, np.int64)  # gather row 0 of phase base
                        slots[ph][slot: slot + g] = arr
                # else: dummy group -> slots stay 0, off stays NEG
                col += 1
                slot += g
        assert col == cap and slot == n_sub * SUB

        # wrapped int16 index tensors [2, n_sub, 128, SUB//16]
        gidx = np.zeros((2, n_sub, 128, SUB // 16), np.int16)
        ii = np.arange(SUB)
        p16 = ii % 16
        c16 = ii // 16
        for ph in range(2):
            s = slots[ph].reshape(n_sub, SUB).astype(np.int16)
            for g16 in range(8):
                gidx[ph, :, g16 * 16 + p16, c16] = s.T
        cores.append(dict(col_node=col_node, gidx=gidx, off=off))

    return dict(cores=cores, sub_g=sub_g, n_sub=n_sub, cap=cap, capP=capP)


def _build_program(n_sub, sub_g, capP):
    import concourse.bass as bass
    import concourse.bacc as bacc
    import concourse.mybir as mybir
    import concourse.tile as tile
    from concourse import library_config

    bf16 = mybir.dt.bfloat16
    f32 = mybir.dt.float32
    i16 = mybir.dt.int16
    AX = mybir.AxisListType.X
    MAX = mybir.AluOpType.max

    nc = bacc.Bacc(None, target_bir_lowering=False, num_swdge_queues=4)
    xb = nc.dram_tensor("xb", [N_NODES, D_IN], bf16, kind="ExternalInput")
    gidx = nc.dram_tensor("gidx", [2, n_sub, 128, SUB // 16], i16, kind="ExternalInput")
    wfc = nc.dram_tensor("wfc", [D_IN, D_HID], bf16, kind="ExternalInput")
    fcb = nc.dram_tensor("fcb", [128, 4], f32, kind="ExternalInput")
    offs = nc.dram_tensor("offs", [2, 128, capP], bf16, kind="ExternalInput")
    xt = nc.dram_tensor("xt", [128, capP], bf16, kind="ExternalInput")
    wout = nc.dram_tensor("wout", [128, 5 * D_OUT], bf16, kind="ExternalInput")
    outp = nc.dram_tensor("out", [capP, D_OUT], f32, kind="ExternalOutput")

    with tile.TileContext(nc) as tc:
        with tc.tile_pool(name="const", bufs=1) as cpool, \
             tc.tile_pool(name="io", bufs=4) as iopool, \
             tc.tile_pool(name="work", bufs=6) as wpool:
            nc.gpsimd.load_library(library_config.mlp)
            wfc_sb = cpool.tile([128, D_HID], bf16)
            nc.sync.dma_start(wfc_sb[:], wfc[:])
            fcb_sb = cpool.tile([128, 4], f32)
            nc.sync.dma_start(fcb_sb[:], fcb[:])
            agg = [cpool.tile([128, 4, capP], bf16, name=f"agg{p}") for p in range(2)]

            with tc.tile_pool(name="mm", bufs=8, space="PSUM") as mmpool:
                drain = 0
                for ph in range(2):
                    colpos = 0
                    for s in range(n_sub):
                        g = sub_g[s]
                        ng = SUB // g
                        idx_sb = iopool.tile([128, SUB // 16], i16, tag="idx")
                        nc.sync.dma_start(idx_sb[:], gidx[ph, s, :, :])
                        rhs = iopool.tile([128, 1, SUB], bf16, tag="rhs")
                        nc.gpsimd.dma_gather(
                            out_ap=rhs[:],
                            in_ap=xb[SPLIT:, :] if ph else xb[:SPLIT, :],
                            idxs_ap=idx_sb[:],
                            num_idxs=SUB,
                            num_idxs_reg=SUB,
                            elem_size=D_IN,
                            transpose=True,
                            queue_num=s % 4,
                        )
                        for h in range(4):
                            pm = mmpool.tile([128, ng, g], f32, tag="mm")
                            nc.tensor.matmul(
                                out=pm[:],
                                lhsT=wfc_sb[:, h * 128:(h + 1) * 128],
                                rhs=rhs[:, 0, :],
                                start=True, stop=True,
                            )
                            out_ap = agg[ph][:, h, colpos:colpos + ng]
                            if drain % 4 == 0 or g == 1:
                                nc.vector.tensor_reduce(
                                    out=out_ap, in_=pm[:], axis=AX, op=MAX)
                            else:
                                vt = wpool.tile([128, ng, g], bf16, tag="vt")
                                nc.scalar.copy(out=vt[:], in_=pm[:])
                                w = g
                                while w > 2:
                                    half = w // 2
                                    nc.vector.tensor_tensor(
                                        out=vt[:, :, :half], in0=vt[:, :, :half],
                                        in1=vt[:, :, half:w], op=MAX)
                                    w = half
                                nc.vector.tensor_tensor(
                                    out=out_ap,
                                    in0=vt[:, :, 0:1].squeeze(-1),
                                    in1=vt[:, :, 1:2].squeeze(-1),
                                    op=MAX)
                            drain += 1
                        colpos += ng

                # merge phases: agg = max(agg_lo + off_lo, agg_hi + off_hi)
                off_sb = [cpool.tile([128, capP], bf16, name=f"off{p}") for p in range(2)]
                for p in range(2):
                    nc.sync.dma_start(off_sb[p][:], offs[p, :, :])
                    for h in range(4):
                        nc.vector.tensor_tensor(
                            out=agg[p][:, h, :], in0=agg[p][:, h, :],
                            in1=off_sb[p][:], op=mybir.AluOpType.add)
                for h in range(4):
                    nc.vector.tensor_tensor(
                        out=agg[0][:, h, :], in0=agg[0][:, h, :],
                        in1=agg[1][:, h, :], op=MAX)
                    # bias + relu (per-partition bias, exact on comparisons)
                    nc.scalar.activation(
                        out=agg[0][:, h, :], in_=agg[0][:, h, :],
                        func=mybir.ActivationFunctionType.Relu,
                        bias=fcb_sb[:, h:h + 1], scale=1.0)

            # final data-parallel matmul over node chunks
            xt_sb = cpool.tile([128, capP], bf16)
            nc.sync.dma_start(xt_sb[:], xt[:])
            wout_sb = cpool.tile([128, 5 * D_OUT], bf16)
            nc.sync.dma_start(wout_sb[:], wout[:])
            with tc.tile_pool(name="fin", bufs=4, space="PSUM") as finpool:
                for m in range(capP // 128):
                    pm2 = finpool.tile([128, D_OUT], f32, tag="fmm")
                    for c in range(5):
                        lhsT = (xt_sb[:, m * 128:(m + 1) * 128] if c == 0
                                else agg[0][:, c - 1, m * 128:(m + 1) * 128])
                        nc.tensor.matmul(
                            out=pm2[:], lhsT=lhsT,
                            rhs=wout_sb[:, c * D_OUT:(c + 1) * D_OUT],
                            start=(c == 0), stop=(c == 4))
                    osb = wpool.tile([128, D_OUT], f32, tag="osb")
                    nc.scalar.copy(out=osb[:], in_=pm2[:])
                    nc.sync.dma_start(outp[m * 128:(m + 1) * 128, :], osb[:])

    nc.finalize()
    return nc


def kernel(input_matrix, fc_w, fc_b, weights_matrix, adjacency_coo_matrix):
    global _compiled, LAST_RES
    from concourse.bass_utils import run_bass_kernel_spmd

    X = np.asarray(input_matrix, np.float32)
    Wfc = np.asarray(fc_w, np.float32)
    bfc = np.asarray(fc_b, np.float32)
    Wout = np.asarray(weights_matrix, np.float32)

    host = _build_host_structures(np.asarray(adjacency_coo_matrix))
    n_sub, sub_g, capP = host["n_sub"], host["sub_g"], host["capP"]

    if _compiled is None:
        _compiled = _build_program(n_sub, sub_g, capP)
    nc = _compiled

    Xb = X.astype(ml_dtypes.bfloat16)
    wfc_in = Wfc.astype(ml_dtypes.bfloat16)
    fcb_in = np.ascontiguousarray(bfc.reshape(4, 128).T.astype(np.float32))
    # wout packed [128, 5*128]: chunk c rows c*128..c*128+127
    wout_in = np.ascontiguousarray(
        Wout.reshape(5, 128, D_OUT).transpose(1, 0, 2).reshape(128, 5 * D_OUT)
    ).astype(ml_dtypes.bfloat16)

    in_maps = []
    for k in range(NCORES):
        hc = host["cores"][k]
        col_node = hc["col_node"]
        safe = np.maximum(col_node, 0)
        xt_in = Xb[np.minimum(safe, N_NODES - 1)] * (col_node >= 0)[:, None].astype(np.float32)
        xt_in = np.ascontiguousarray(xt_in.T.astype(ml_dtypes.bfloat16))
        off_in = np.ascontiguousarray(
            np.broadcast_to(hc["off"][:, None, :], (2, 128, capP))
        ).astype(ml_dtypes.bfloat16)
        in_maps.append({
            "xb": Xb,
            "gidx": hc["gidx"],
            "wfc": wfc_in,
            "fcb": fcb_in,
            "offs": off_in,
            "xt": xt_in,
            "wout": wout_in,
        })

    res = run_bass_kernel_spmd(nc, in_maps, list(range(NCORES)))
    LAST_RES = res

    out_full = np.zeros((N_NODES, D_OUT), np.float32)
    for k in range(NCORES):
        got = np.asarray(res.results[k]["out"], np.float32)
        col_node = host["cores"][k]["col_node"]
        valid = col_node >= 0
        out_full[col_node[valid]] = got[valid]
    return out_full


# revision 41
# speedup vs baseline: 1.0532x; 1.0532x over previous
"""MaxPoolAggregator GNN kernel for 8 Trainium2 NeuronCores.

Reference computation:
    H = relu(X[trg] @ fc_w + fc_b)  per edge           [E, 512]
    agg = clamp0(segment_max(H, src))                  [N, 512]
    out = concat([X, agg], 1) @ weights_matrix         [N, 128]

Strategy (src-partitioned, no cross-core reduction):
  - Each core owns a contiguous range of 6272 src nodes and all their edges.
  - Host sorts edges by src, buckets each node by next_pow2(max(lo_deg, hi_deg))
    where lo/hi split the target-id space at 25088 (dma_gather int16 limit).
  - Per 512-slot subtile: dma_gather(transpose) pulls bf16 X rows of the edge
    targets directly in feature-major layout; PE matmuls against fc_w chunks
    (bf16, f32 PSUM); the grouped segment max runs on DVE (tensor_reduce from
    PSUM) or ACT-copy + DVE bf16 max-tree (drain bandwidth split).
  - Pads duplicate a real neighbor (max-idempotent); phase-empty groups are
    knocked out with a -1e30 column offset before the cross-phase max merge.
  - max commutes with relu and the +bias is per-feature, so bias+relu runs
    once per node on ACT after the reduce; empty nodes relu(-1e30+b) = 0,
    matching the reference's zero-fill clamp.
  - Final: out = [X^T; agg^T]^T @ wout as 5 accumulated K=128 matmuls per
    128-node chunk, written node-major straight to DRAM.
"""
import sys
import os

sys.path.insert(0, "/opt/trn_rl_repo")

import numpy as np
import ml_dtypes

N_NODES = 50000
N_EDGES = 800000
D_IN = 128
D_HID = 512
D_OUT = 128
NCORES = 8
NPC = 6272            # nodes per core (50176 padded / 8)
SPLIT = 25088         # target-range split for int16 gather indices
SUB = 512             # slots per gather/reduce subtile
def _make_buckets(ratio=2.0):
    b = [1]
    while b[-1] < 512:
        b.append(min(512, max(b[-1] + 1, int(b[-1] * ratio))))
    return b


BUCKETS = _make_buckets()
NEG = -1.0e30

_compiled = None
LAST_RES = None


def _build_host_structures(adjacency):
    """Sort edges by src, bucket nodes, build per-core slot/index streams."""
    src = np.asarray(adjacency[0], dtype=np.int64)
    trg = np.asarray(adjacency[1], dtype=np.int64)
    order = np.argsort(src, kind="stable")
    src_s = src[order]
    trg_s = trg[order]
    deg = np.bincount(src, minlength=N_NODES).astype(np.int64)
    rowptr = np.zeros(N_NODES + 1, np.int64)
    np.cumsum(deg, out=rowptr[1:])
    # per-node lo/hi degree (trg < SPLIT vs >=)
    is_lo = (trg_s < SPLIT).astype(np.int64)
    lo_cum = np.zeros(N_EDGES + 1, np.int64)
    np.cumsum(is_lo, out=lo_cum[1:])
    d_lo = lo_cum[rowptr[1:]] - lo_cum[rowptr[:-1]]
    d_hi = deg - d_lo
    assert deg.max() <= 512, f"degree {deg.max()} exceeds supported 512"

    dmax = np.maximum(np.maximum(d_lo, d_hi), 1)
    bidx = np.searchsorted(BUCKETS, dmax, side="left")  # BUCKETS[bidx] >= dmax

    cores = []
    # per-bucket subtile counts must be equal across cores (SPMD): take max
    per_core_nodes = [np.arange(k * NPC, min((k + 1) * NPC, N_NODES)) for k in range(NCORES)]
    n_sub_per_bucket = []
    for gi, g in enumerate(BUCKETS):
        counts = [(bidx[nodes] == gi).sum() for nodes in per_core_nodes]
        gps = SUB // g
        n_sub_per_bucket.append(
            max((int(c) + gps - 1) // gps for c in counts) if max(counts) else 0
        )
    n_sub = sum(n_sub_per_bucket)
    # subtile -> bucket size table (same for all cores)
    sub_g = []
    for gi, g in enumerate(BUCKETS):
        sub_g += [g] * n_sub_per_bucket[gi]
    cap = sum(SUB // g for g in sub_g)          # total output columns
    capP = ((cap + 127) // 128) * 128           # padded for final matmul

    for k in range(NCORES):
        nodes = per_core_nodes[k]
        col_node = np.full(capP, -1, np.int64)  # column -> global node id
        # slot target streams per phase
        slots = {0: np.zeros(n_sub * SUB, np.int64), 1: np.zeros(n_sub * SUB, np.int64)}
        off = np.zeros((2, capP), np.float32)   # -1e30 for empty/trash columns
        off[:, :] = NEG                          # default: trash
        col = 0
        slot = 0
        for gi, g in enumerate(BUCKETS):
            if n_sub_per_bucket[gi] == 0:
                continue
            sel = nodes[bidx[nodes] == gi]
            gps = SUB // g                  # groups per subtile
            assert len(sel) <= n_sub_per_bucket[gi] * gps
            tail = SUB - gps * g            # subtile tail pad (non-pow2 g)
            cap_groups = n_sub_per_bucket[gi] * gps
            for i in range(cap_groups):
                if i < len(sel):
                    n = int(sel[i])
                    col_node[col] = n
                    lo0, hi0 = rowptr[n], rowptr[n + 1]
                    tt = trg_s[lo0:hi0]
                    tlo = tt[tt < SPLIT]
                    thi = tt[tt >= SPLIT] - SPLIT
                    for ph, tp in ((0, tlo), (1, thi)):
                        if len(tp):
                            arr = np.empty(g, np.int64)
                            arr[: len(tp)] = tp
                            arr[len(tp):] = tp[0]
                            off[ph, col] = 0.0
                        else:
                            arr = np.zeros(g, np.int64)  # gather row 0 of phase base
                        slots[ph][slot: slot + g] = arr
                # else: dummy group -> slots stay 0, off stays NEG
                col += 1
                slot += g
                if (i + 1) % gps == 0:
                    slot += tail
        assert col == cap and slot == n_sub * SUB

        # wrapped int16 index tensors [2, n_sub, 128, SUB//16]
        gidx = np.zeros((2, n_sub, 128, SUB // 16), np.int16)
        ii = np.arange(SUB)
        p16 = ii % 16
        c16 = ii // 16
        for ph in range(2):
            s = slots[ph].reshape(n_sub, SUB).astype(np.int16)
            for g16 in range(8):
                gidx[ph, :, g16 * 16 + p16, c16] = s.T
        cores.append(dict(col_node=col_node, gidx=gidx, off=off))

    return dict(cores=cores, sub_g=sub_g, n_sub=n_sub, cap=cap, capP=capP)


def _build_program(n_sub, sub_g, capP):
    import concourse.bass as bass
    import concourse.bacc as bacc
    import concourse.mybir as mybir
    import concourse.tile as tile
    from concourse import library_config

    bf16 = mybir.dt.bfloat16
    f32 = mybir.dt.float32
    i16 = mybir.dt.int16
    AX = mybir.AxisListType.X
    MAX = mybir.AluOpType.max

    nc = bacc.Bacc(None, target_bir_lowering=False, num_swdge_queues=4)
    xb = nc.dram_tensor("xb", [N_NODES, D_IN], bf16, kind="ExternalInput")
    gidx = nc.dram_tensor("gidx", [2, n_sub, 128, SUB // 16], i16, kind="ExternalInput")
    wfc = nc.dram_tensor("wfc", [D_IN, D_HID], bf16, kind="ExternalInput")
    fcb = nc.dram_tensor("fcb", [128, 4], f32, kind="ExternalInput")
    offs = nc.dram_tensor("offs", [2, 128, capP], bf16, kind="ExternalInput")
    xt = nc.dram_tensor("xt", [128, capP], bf16, kind="ExternalInput")
    wout = nc.dram_tensor("wout", [128, 5 * D_OUT], bf16, kind="ExternalInput")
    outp = nc.dram_tensor("out", [capP, D_OUT], f32, kind="ExternalOutput")

    with tile.TileContext(nc) as tc:
        with tc.tile_pool(name="const", bufs=1) as cpool, \
             tc.tile_pool(name="io", bufs=4) as iopool, \
             tc.tile_pool(name="work", bufs=6) as wpool:
            nc.gpsimd.load_library(library_config.mlp)
            wfc_sb = cpool.tile([128, D_HID], bf16)
            nc.sync.dma_start(wfc_sb[:], wfc[:])
            fcb_sb = cpool.tile([128, 4], f32)
            nc.sync.dma_start(fcb_sb[:], fcb[:])
            agg = [cpool.tile([128, 4, capP], bf16, name=f"agg{p}") for p in range(2)]

            with tc.tile_pool(name="mm", bufs=8, space="PSUM") as mmpool:
                drain = 0
                for ph in range(2):
                    colpos = 0
                    for s in range(n_sub):
                        g = sub_g[s]
                        ng = SUB // g
                        idx_sb = iopool.tile([128, SUB // 16], i16, tag="idx")
                        nc.sync.dma_start(idx_sb[:], gidx[ph, s, :, :])
                        rhs = iopool.tile([128, 1, SUB], bf16, tag="rhs")
                        nc.gpsimd.dma_gather(
                            out_ap=rhs[:],
                            in_ap=xb[SPLIT:, :] if ph else xb[:SPLIT, :],
                            idxs_ap=idx_sb[:],
                            num_idxs=SUB,
                            num_idxs_reg=SUB,
                            elem_size=D_IN,
                            transpose=True,
                            queue_num=s % 4,
                        )
                        for h in range(4):
                            pm = mmpool.tile([128, ng, g], f32, tag="mm")
                            nc.tensor.matmul(
                                out=pm[:],
                                lhsT=wfc_sb[:, h * 128:(h + 1) * 128],
                                rhs=rhs[:, 0, :ng * g],
                                start=True, stop=True,
                            )
                            out_ap = agg[ph][:, h, colpos:colpos + ng]
                            if drain % 4 == 0 or g == 1:
                                nc.vector.tensor_reduce(
                                    out=out_ap, in_=pm[:], axis=AX, op=MAX)
                            else:
                                vt = wpool.tile([128, ng, g], bf16, tag="vt")
                                nc.scalar.copy(out=vt[:], in_=pm[:])
                                w = g
                                while w > 2:
                                    fl = w // 2
                                    ce = w - fl
                                    # fold fl pairs; odd middle stays in place
                                    nc.vector.tensor_tensor(
                                        out=vt[:, :, :fl], in0=vt[:, :, :fl],
                                        in1=vt[:, :, ce:w], op=MAX)
                                    w = ce
                                i1 = 1 if w == 2 else 0
                                nc.vector.tensor_tensor(
                                    out=out_ap,
                                    in0=vt[:, :, 0:1].squeeze(-1),
                                    in1=vt[:, :, i1:i1 + 1].squeeze(-1),
                                    op=MAX)
                            drain += 1
                        colpos += ng

                # merge phases: agg = max(agg_lo + off_lo, agg_hi + off_hi)
                off_sb = [cpool.tile([128, capP], bf16, name=f"off{p}") for p in range(2)]
                for p in range(2):
                    nc.sync.dma_start(off_sb[p][:], offs[p, :, :])
                    for h in range(4):
                        nc.vector.tensor_tensor(
                            out=agg[p][:, h, :], in0=agg[p][:, h, :],
                            in1=off_sb[p][:], op=mybir.AluOpType.add)
                for h in range(4):
                    nc.vector.tensor_tensor(
                        out=agg[0][:, h, :], in0=agg[0][:, h, :],
                        in1=agg[1][:, h, :], op=MAX)
                    # bias + relu (per-partition bias, exact on comparisons)
                    nc.scalar.activation(
                        out=agg[0][:, h, :], in_=agg[0][:, h, :],
                        func=mybir.ActivationFunctionType.Relu,
                        bias=fcb_sb[:, h:h + 1], scale=1.0)

            # final data-parallel matmul over node chunks
            xt_sb = cpool.tile([128, capP], bf16)
            nc.sync.dma_start(xt_sb[:], xt[:])
            wout_sb = cpool.tile([128, 5 * D_OUT], bf16)
            nc.sync.dma_start(wout_sb[:], wout[:])
            with tc.tile_pool(name="fin", bufs=4, space="PSUM") as finpool:
                for m in range(capP // 128):
                    pm2 = finpool.tile([128, D_OUT], f32, tag="fmm")
                    for c in range(5):
                        lhsT = (xt_sb[:, m * 128:(m + 1) * 128] if c == 0
                                else agg[0][:, c - 1, m * 128:(m + 1) * 128])
                        nc.tensor.matmul(
                            out=pm2[:], lhsT=lhsT,
                            rhs=wout_sb[:, c * D_OUT:(c + 1) * D_OUT],
                            start=(c == 0), stop=(c == 4))
                    osb = wpool.tile([128, D_OUT], f32, tag="osb")
                    nc.scalar.copy(out=osb[:], in_=pm2[:])
                    nc.sync.dma_start(outp[m * 128:(m + 1) * 128, :], osb[:])

    nc.finalize()
    return nc


def kernel(input_matrix, fc_w, fc_b, weights_matrix, adjacency_coo_matrix):
    global _compiled, LAST_RES
    from concourse.bass_utils import run_bass_kernel_spmd

    X = np.asarray(input_matrix, np.float32)
    Wfc = np.asarray(fc_w, np.float32)
    bfc = np.asarray(fc_b, np.float32)
    Wout = np.asarray(weights_matrix, np.float32)

    host = _build_host_structures(np.asarray(adjacency_coo_matrix))
    n_sub, sub_g, capP = host["n_sub"], host["sub_g"], host["capP"]

    if _compiled is None:
        _compiled = _build_program(n_sub, sub_g, capP)
    nc = _compiled

    Xb = X.astype(ml_dtypes.bfloat16)
    wfc_in = Wfc.astype(ml_dtypes.bfloat16)
    fcb_in = np.ascontiguousarray(bfc.reshape(4, 128).T.astype(np.float32))
    # wout packed [128, 5*128]: chunk c rows c*128..c*128+127
    wout_in = np.ascontiguousarray(
        Wout.reshape(5, 128, D_OUT).transpose(1, 0, 2).reshape(128, 5 * D_OUT)
    ).astype(ml_dtypes.bfloat16)

    in_maps = []
    for k in range(NCORES):
        hc = host["cores"][k]
        col_node = hc["col_node"]
        safe = np.maximum(col_node, 0)
        xt_in = Xb[np.minimum(safe, N_NODES - 1)] * (col_node >= 0)[:, None].astype(np.float32)
        xt_in = np.ascontiguousarray(xt_in.T.astype(ml_dtypes.bfloat16))
        off_in = np.ascontiguousarray(
            np.broadcast_to(hc["off"][:, None, :], (2, 128, capP))
        ).astype(ml_dtypes.bfloat16)
        in_maps.append({
            "xb": Xb,
            "gidx": hc["gidx"],
            "wfc": wfc_in,
            "fcb": fcb_in,
            "offs": off_in,
            "xt": xt_in,
            "wout": wout_in,
        })

    res = run_bass_kernel_spmd(nc, in_maps, list(range(NCORES)))
    LAST_RES = res

    out_full = np.zeros((N_NODES, D_OUT), np.float32)
    for k in range(NCORES):
        got = np.asarray(res.results[k]["out"], np.float32)
        col_node = host["cores"][k]["col_node"]
        valid = col_node >= 0
        out_full[col_node[valid]] = got[valid]
    return out_full
